# revision 22
# baseline (speedup 1.0000x reference)
"""FM (factorization machine) layer kernel for Trainium2, 8 NeuronCores.

Strategy: data-parallel over the batch (512 rows/core), no collectives.
The gather bottleneck is SWDGE descriptor generation on the Pool engine
(~9 ns/descriptor, single queue). v2 replaces the 104 indirect_dma_start
calls per core (one per tile*field, 128 descriptors each) with 26
InstDMAGatherAnt gathers (one per sparse field, 512 descriptors each)
spread over the 4 SWDGE queues, which run descriptor generation in
parallel (~3x measured).

dma_gather indices are SIGNED int16 (0..32767), but a field has 40000
categories. Workaround: the table stores row PAIRS (512 B descriptors:
rows 2k,2k+1 of 128 fp16 each), the gather index is i//2 < 20000, and a
per-slot parity mask selects the wanted half on the vector engine as
sel = A + m*(B-A) with the mask broadcast (stride-0) along the row dim.
(InstCopyPredicated would be 2 ops instead of 3 but the NEFF backend
rejects it; stride-0 APs on plain DVE ops compile fine.)

Algebra (per batch row b, j indexes the K latent dims):
    out = w0 + dense@w_d + sum_s w[i_bs] + 0.5*(sum_j sv_j^2 - sum_j s2v2_j)
    sv  = dense@Vt_d + sum_s Vt[i_bs]
    sum_j s2v2_j = dense^2 @ csq + sum_s ||Vt[i_bs]||^2
Per-row scalars fold into table column 64: c[f] = w[f] - 0.5*||Vt[f]||^2
(fp16, like the latent vectors; dense/w0 terms stay f32 via one [27,65]
matmul), so the output per tile is ACT Square-accumulate + one add.
"""
import numpy as np

import concourse.bass as bass
import concourse.bacc as bacc
import concourse.mybir as mybir
import concourse.tile as tile
from concourse import bass_utils

NUM_DENSE = 13
NUM_SPARSE = 26
FEAT_NUM = 40000
K = 64
BATCH = 4096
N_CORES = 8
BPC = BATCH // N_CORES  # 512 batch rows per core
P = 128
NT = BPC // P  # 4 tiles per core
ROW = K + 1  # 64 latent dims + combined scalar column
ROWE = 128  # fp16 elements per padded table row
PAIR_E = 2 * ROWE  # gathered element: a pair of rows
PAIRS = FEAT_NUM // 2  # 20000 pairs per field
CWT = P // 16  # idx columns per (field, tile) gather (16-partition wrap)
CDIM = 2 * NUM_DENSE + 1  # dense | dense^2 | ones
NQ = 4  # SWDGE queues

TRACE = False  # test harness flips this to get an NTFF profile
LAST = {}  # test harness reads LAST["res"] for exec_time_ns

_nc_cache = []


def _build():
    f32 = mybir.dt.float32
    f16 = mybir.dt.float16
    i16 = mybir.dt.int16
    nc = bacc.Bacc(
        "TRN2",
        target_bir_lowering=False,
        debug=False,
        num_devices=N_CORES,
        num_swdge_queues=NQ,
        # descriptor carveout: default 16KB = 1024 descs/queue stalls the
        # Pool engine after 2 in-flight 512-desc gathers per queue; 64KB
        # lets a queue's whole gather backlog generate without waiting
        # for DMA-completion reclaim.
        dynamic_dma_scratch_size=65536,
    )
    tablep_d = nc.dram_tensor(
        "tablep", [NUM_SPARSE * PAIRS, PAIR_E], f16, kind="ExternalInput"
    ).ap()
    idx_d = nc.dram_tensor(
        "idx", [P, NUM_SPARSE * NT * CWT], i16, kind="ExternalInput"
    ).ap()
    par_d = nc.dram_tensor(
        "par", [P, NT * NUM_SPARSE * ROW], f16, kind="ExternalInput"
    ).ap()
    lhs_d = nc.dram_tensor("lhs", [CDIM, BPC], f32, kind="ExternalInput").ap()
    rhs_d = nc.dram_tensor("rhs", [CDIM, ROW], f32, kind="ExternalInput").ap()
    y_d = nc.dram_tensor("y", [BPC, 1], f32, kind="ExternalOutput").ap()

    with tile.TileContext(nc) as tc:
        with (
            tc.tile_pool(name="gp", bufs=1) as gp,
            tc.tile_pool(name="sp", bufs=2) as sp,
            tc.tile_pool(name="cp", bufs=1) as cp,
            tc.tile_pool(name="pp", bufs=2, space="PSUM") as pp,
        ):
            # idx first: the gathers wait only on it
            idx_sb = cp.tile([P, NUM_SPARSE * NT * CWT], i16)
            nc.sync.dma_start(idx_sb[:], idx_d[:, :])
            rhs_sb = cp.tile([CDIM, ROW], f32)
            nc.sync.dma_start(rhs_sb[:], rhs_d[:, :])
            lhs_sb = cp.tile([CDIM, BPC], f32)
            nc.sync.dma_start(lhs_sb[:], lhs_d[:, :])
            par_sb = cp.tile([P, NT, NUM_SPARSE, ROW], f16)
            nc.sync.dma_start(par_sb[:], par_d[:, :])

            # dense/w0 matmuls depend only on the tiny lhs/rhs loads —
            # issue them before the gathers so PE runs during the gather
            # phase and the post-tree epilogue is short.
            psum_all = pp.tile([P, NT, ROW], f32, space="PSUM")
            for t in range(NT):
                nc.tensor.matmul(
                    out=psum_all[:, t, :],
                    lhsT=lhs_sb[:, t * P : (t + 1) * P],
                    rhs=rhs_sb[:],
                    start=True,
                    stop=True,
                )

            reg_ni = nc.gpsimd.to_reg(BPC)
            g = gp.tile([P, NUM_SPARSE, NT, PAIR_E], f16)
            for s in range(NUM_SPARSE):
                nc.gpsimd.dma_gather(
                    out_ap=g[:, s, :, :],
                    in_ap=tablep_d[s * PAIRS : (s + 1) * PAIRS, :],
                    idxs_ap=idx_sb[:, s * NT * CWT : (s + 1) * NT * CWT],
                    num_idxs=BPC,
                    num_idxs_reg=reg_ni,
                    elem_size=PAIR_E,
                    queue_num=s % NQ,
                )

            # parity select per tile: the table's second half stores
            # (odd - even), so sel = even + m*delta is 2 ops; the mask is
            # host-expanded to full [26, ROW] (contiguous reads beat the
            # stride-0 broadcast by ~2x). 2-free-dim APs only (3-free-dim
            # DVE ops and copy_predicated miscompile); sel is one
            # contiguous buffer so the tree fuses across tiles.
            TR = NT * ROW
            sel = gp.tile([P, NUM_SPARSE, TR], f16)
            dm0 = sp.tile([P, NUM_SPARSE, ROW], f16, tag="dm0")
            dm1 = sp.tile([P, NUM_SPARSE, ROW], f16, tag="dm1")
            dm2 = sp.tile([P, NUM_SPARSE, ROW], f16, tag="dm2")
            dm3 = sp.tile([P, NUM_SPARSE, ROW], f16, tag="dm3")
            dms = [dm0, dm1, dm2, dm3]
            # interleave independent muls between dependent mul->add pairs
            # so the engine queue never drains on a dependency.
            def sel_add(t):
                nc.vector.tensor_add(
                    sel[:, :, t * ROW : (t + 1) * ROW],
                    g[:, :, t, 0:ROW],
                    dms[t][:],
                )

            def sel_mul(t):
                nc.vector.tensor_mul(
                    dms[t][:], g[:, :, t, ROWE : ROWE + ROW], par_sb[:, t, :, :]
                )

            sel_mul(0)
            sel_mul(1)
            sel_add(0)
            sel_mul(2)
            sel_add(1)
            sel_mul(3)
            sel_add(2)
            sel_add(3)
            # field-sum tree over 26 rows, fused across tiles:
            # 26->13->6(+1)->3->1; fp16 through level 2, f32 after.
            h16 = gp.tile([P, 13, TR], f16)
            q16 = gp.tile([P, 6, TR], f16)
            h = gp.tile([P, 3, TR], f32)
            w2 = gp.tile([P, 2, TR], f32)
            nc.vector.tensor_add(h16[:, 0:13], sel[:, 0:13], sel[:, 13:26])
            nc.vector.tensor_add(q16[:, 0:6], h16[:, 0:6], h16[:, 6:12])
            nc.vector.tensor_add(h[:, 0:3], q16[:, 0:3], q16[:, 3:6])
            nc.vector.tensor_add(w2[:, 0:1], h[:, 0:1], h[:, 1:2])
            nc.vector.tensor_add(w2[:, 1:2], w2[:, 0:1], h[:, 2:3])
            nc.vector.tensor_add(h[:, 0:1], w2[:, 1:2], h16[:, 12:13])
            # epilogue, fused across tiles: tot = tree + psum (one op),
            # then per-tile ACT square-accumulate, one final add, 4 DMAs.
            tot_all = sp.tile([P, NT, ROW], f32, tag="tot")
            nc.vector.tensor_add(
                tot_all[:, :, :], h[:, 0, :], psum_all[:, :, :]
            )
            acc_all = sp.tile([P, NT], f32, tag="acc")
            scratch = sp.tile([P, K], f32, tag="scr")
            for t in range(NT):
                nc.scalar.activation(
                    out=scratch[:],
                    in_=tot_all[:, t, 0:K],
                    func=mybir.ActivationFunctionType.Square,
                    scale=0.7071067811865476,
                    accum_out=acc_all[:, t : t + 1],
                )
            o_all = sp.tile([P, NT], f32, tag="o")
            nc.vector.tensor_add(o_all[:], acc_all[:], tot_all[:, :, K])
            for t in range(NT):
                nc.sync.dma_start(
                    y_d[t * P : (t + 1) * P, :], o_all[:, t : t + 1]
                )
    nc.compile()
    return nc


def _prepare(dense, sparse, w0, w, V):
    sp = sparse.astype(np.int64)  # [B, 26] category per field
    pair = (sp >> 1).astype(np.int16)  # [B, 26] pair index < 20000
    parity = (sp & 1).astype(np.float16)

    Vt = np.ascontiguousarray(V.T)  # [F, K] f32
    Vs = Vt[NUM_DENSE:]  # sparse-field rows [26*40000, K]
    ws = w[NUM_DENSE:, 0]
    rows = np.zeros((NUM_SPARSE * FEAT_NUM, ROWE), dtype=np.float16)
    rows[:, :K] = Vs.astype(np.float16)
    rows[:, K] = (ws - 0.5 * (Vs * Vs).sum(axis=1)).astype(np.float16)
    # pair layout [even | odd - even]: the on-chip parity select becomes
    # sel = even + m*delta (2 ops instead of 3)
    pairs = rows.reshape(NUM_SPARSE * PAIRS, 2, ROWE)
    tablep = np.empty((NUM_SPARSE * PAIRS, PAIR_E), dtype=np.float16)
    tablep[:, :ROWE] = pairs[:, 0]
    tablep[:, ROWE:] = pairs[:, 1] - pairs[:, 0]

    lhs = np.concatenate(
        [dense.T, dense.T**2, np.ones((1, BATCH), np.float32)], axis=0
    ).astype(np.float32)  # [27, BATCH]
    rhs = np.zeros((CDIM, ROW), dtype=np.float32)
    rhs[0:NUM_DENSE, 0:K] = Vt[:NUM_DENSE]
    rhs[0:NUM_DENSE, K] = w[:NUM_DENSE, 0]
    rhs[NUM_DENSE : 2 * NUM_DENSE, K] = -0.5 * (Vt[:NUM_DENSE] ** 2).sum(axis=1)
    rhs[2 * NUM_DENSE, K] = np.asarray(w0).reshape(-1)[0]
    return pair, parity, tablep, lhs, rhs


def kernel(dense_inputs, sparse_inputs, w0, w, V):
    dense = np.asarray(dense_inputs, dtype=np.float32)
    sparse = np.asarray(sparse_inputs)
    w0 = np.asarray(w0, dtype=np.float32)
    w = np.asarray(w, dtype=np.float32)
    V = np.asarray(V, dtype=np.float32)

    if not _nc_cache:
        _nc_cache.append(_build())
    nc = _nc_cache[0]

    pair, parity, tablep, lhs, rhs = _prepare(dense, sparse, w0, w, V)

    in_maps = []
    for c in range(N_CORES):
        rows = slice(c * BPC, (c + 1) * BPC)
        pair_c = pair[rows]  # [512, 26]
        # idx16[p, (s*NT+t)*CWT + cw] = pair_c[t*128 + cw*16 + p%16, s],
        # 16-row groups replicated across the 8 Q7 cores.
        wrapped = pair_c.reshape(NT, CWT, 16, NUM_SPARSE).transpose(
            2, 3, 0, 1
        )  # [16, 26, NT, CWT]
        idx16 = np.tile(wrapped.reshape(16, NUM_SPARSE * NT * CWT), (8, 1))
        # par[p, ((t*26)+s)*ROW + e] = parity of batch row t*128+p, field s
        # (mask pre-expanded along the ROW dim for contiguous DVE reads)
        par_c = parity[rows].reshape(NT, P, NUM_SPARSE).transpose(1, 0, 2)
        par_exp = np.repeat(
            par_c.reshape(P, NT, NUM_SPARSE, 1), ROW, axis=3
        )  # [128, NT, 26, ROW]
        in_maps.append(
            {
                "tablep": tablep,
                "idx": np.ascontiguousarray(idx16),
                "par": np.ascontiguousarray(
                    par_exp.reshape(P, NT * NUM_SPARSE * ROW)
                ),
                "lhs": np.ascontiguousarray(lhs[:, rows]),
                "rhs": rhs,
            }
        )
    res = bass_utils.run_bass_kernel_spmd(
        nc, in_maps, core_ids=list(range(N_CORES)), trace=TRACE
    )
    LAST["res"] = res
    out = np.concatenate([res.results[c]["y"] for c in range(N_CORES)], axis=0)
    return out.astype(np.float32)


# revision 23
# speedup vs baseline: 1.0345x; 1.0345x over previous
"""FM (factorization machine) layer kernel for Trainium2, 8 NeuronCores.

Strategy: data-parallel over the batch (512 rows/core), no collectives.
The gather bottleneck is SWDGE descriptor generation on the Pool engine
(~9 ns/descriptor, single queue). v2 replaces the 104 indirect_dma_start
calls per core (one per tile*field, 128 descriptors each) with 26
InstDMAGatherAnt gathers (one per sparse field, 512 descriptors each)
spread over the 4 SWDGE queues, which run descriptor generation in
parallel (~3x measured).

dma_gather indices are SIGNED int16 (0..32767), but a field has 40000
categories. Workaround: the table stores row PAIRS (512 B descriptors:
rows 2k,2k+1 of 128 fp16 each), the gather index is i//2 < 20000, and a
per-slot parity mask selects the wanted half on the vector engine as
sel = A + m*(B-A) with the mask broadcast (stride-0) along the row dim.
(InstCopyPredicated would be 2 ops instead of 3 but the NEFF backend
rejects it; stride-0 APs on plain DVE ops compile fine.)

Algebra (per batch row b, j indexes the K latent dims):
    out = w0 + dense@w_d + sum_s w[i_bs] + 0.5*(sum_j sv_j^2 - sum_j s2v2_j)
    sv  = dense@Vt_d + sum_s Vt[i_bs]
    sum_j s2v2_j = dense^2 @ csq + sum_s ||Vt[i_bs]||^2
Per-row scalars fold into table column 64: c[f] = w[f] - 0.5*||Vt[f]||^2
(fp16, like the latent vectors; dense/w0 terms stay f32 via one [27,65]
matmul), so the output per tile is ACT Square-accumulate + one add.
"""
import numpy as np

import concourse.bass as bass
import concourse.bacc as bacc
import concourse.mybir as mybir
import concourse.tile as tile
from concourse import bass_utils

NUM_DENSE = 13
NUM_SPARSE = 26
FEAT_NUM = 40000
K = 64
BATCH = 4096
N_CORES = 8
BPC = BATCH // N_CORES  # 512 batch rows per core
P = 128
NT = BPC // P  # 4 tiles per core
ROW = K + 1  # 64 latent dims + combined scalar column
ROWE = 128  # fp16 elements per padded table row
PAIR_E = 2 * ROWE  # gathered element: a pair of rows
PAIRS = FEAT_NUM // 2  # 20000 pairs per field
CWT = P // 16  # idx columns per (field, tile) gather (16-partition wrap)
CDIM = 2 * NUM_DENSE + 1  # dense | dense^2 | ones
NQ = 4  # SWDGE queues

TRACE = False  # test harness flips this to get an NTFF profile
LAST = {}  # test harness reads LAST["res"] for exec_time_ns

_nc_cache = []


def _build():
    f32 = mybir.dt.float32
    f16 = mybir.dt.float16
    i16 = mybir.dt.int16
    nc = bacc.Bacc(
        "TRN2",
        target_bir_lowering=False,
        debug=False,
        num_devices=N_CORES,
        num_swdge_queues=NQ,
        # descriptor carveout: default 16KB = 1024 descs/queue stalls the
        # Pool engine after 2 in-flight 512-desc gathers per queue; 64KB
        # lets a queue's whole gather backlog generate without waiting
        # for DMA-completion reclaim.
        dynamic_dma_scratch_size=65536,
    )
    tablep_d = nc.dram_tensor(
        "tablep", [NUM_SPARSE * PAIRS, PAIR_E], f16, kind="ExternalInput"
    ).ap()
    idx_d = nc.dram_tensor(
        "idx", [P, NUM_SPARSE * NT * CWT], i16, kind="ExternalInput"
    ).ap()
    par_d = nc.dram_tensor(
        "par", [P, NT * NUM_SPARSE * ROW], f16, kind="ExternalInput"
    ).ap()
    lhs_d = nc.dram_tensor("lhs", [CDIM, BPC], f32, kind="ExternalInput").ap()
    rhs_d = nc.dram_tensor("rhs", [CDIM, ROW], f32, kind="ExternalInput").ap()
    y_d = nc.dram_tensor("y", [BPC, 1], f32, kind="ExternalOutput").ap()

    with tile.TileContext(nc) as tc:
        with (
            tc.tile_pool(name="gp", bufs=1) as gp,
            tc.tile_pool(name="sp", bufs=2) as sp,
            tc.tile_pool(name="cp", bufs=1) as cp,
            tc.tile_pool(name="pp", bufs=2, space="PSUM") as pp,
        ):
            # idx first: the gathers wait only on it
            idx_sb = cp.tile([P, NUM_SPARSE * NT * CWT], i16)
            nc.sync.dma_start(idx_sb[:], idx_d[:, :])
            rhs_sb = cp.tile([CDIM, ROW], f32)
            nc.sync.dma_start(rhs_sb[:], rhs_d[:, :])
            lhs_sb = cp.tile([CDIM, BPC], f32)
            nc.sync.dma_start(lhs_sb[:], lhs_d[:, :])
            par_sb = cp.tile([P, NT, NUM_SPARSE, ROW], f16)
            nc.sync.dma_start(par_sb[:], par_d[:, :])

            # dense/w0 matmuls depend only on the tiny lhs/rhs loads —
            # issue them before the gathers so PE runs during the gather
            # phase and the post-tree epilogue is short.
            psum_all = pp.tile([P, NT, ROW], f32, space="PSUM")
            for t in range(NT):
                nc.tensor.matmul(
                    out=psum_all[:, t, :],
                    lhsT=lhs_sb[:, t * P : (t + 1) * P],
                    rhs=rhs_sb[:],
                    start=True,
                    stop=True,
                )

            reg_ni = nc.gpsimd.to_reg(BPC)
            g = gp.tile([P, NUM_SPARSE, NT, PAIR_E], f16)
            for s in range(NUM_SPARSE):
                nc.gpsimd.dma_gather(
                    out_ap=g[:, s, :, :],
                    in_ap=tablep_d[s * PAIRS : (s + 1) * PAIRS, :],
                    idxs_ap=idx_sb[:, s * NT * CWT : (s + 1) * NT * CWT],
                    num_idxs=BPC,
                    num_idxs_reg=reg_ni,
                    elem_size=PAIR_E,
                    queue_num=s % NQ,
                )

            # parity select per tile: the table's second half stores
            # (odd - even), so sel = even + m*delta is 2 ops; the mask is
            # host-expanded to full [26, ROW] (contiguous reads beat the
            # stride-0 broadcast by ~2x). 2-free-dim APs only (3-free-dim
            # DVE ops and copy_predicated miscompile); sel is one
            # contiguous buffer so the tree fuses across tiles.
            TR = NT * ROW
            sel = gp.tile([P, NUM_SPARSE, TR], f16)
            dm0 = sp.tile([P, NUM_SPARSE, ROW], f16, tag="dm0")
            dm1 = sp.tile([P, NUM_SPARSE, ROW], f16, tag="dm1")
            dm2 = sp.tile([P, NUM_SPARSE, ROW], f16, tag="dm2")
            dm3 = sp.tile([P, NUM_SPARSE, ROW], f16, tag="dm3")
            dms = [dm0, dm1, dm2, dm3]
            # interleave independent muls between dependent mul->add pairs
            # so the engine queue never drains on a dependency.
            def sel_add(t):
                nc.vector.tensor_add(
                    sel[:, :, t * ROW : (t + 1) * ROW],
                    g[:, :, t, 0:ROW],
                    dms[t][:],
                )

            def sel_mul(t):
                nc.vector.tensor_mul(
                    dms[t][:], g[:, :, t, ROWE : ROWE + ROW], par_sb[:, t, :, :]
                )

            sel_mul(0)
            sel_mul(1)
            sel_add(0)
            sel_mul(2)
            sel_add(1)
            sel_mul(3)
            sel_add(2)
            sel_add(3)
            # field-sum tree over 26 rows, fused across tiles:
            # 26->13->6(+1)->3->1; fp16 through level 2, f32 after.
            h16 = gp.tile([P, 13, TR], f16)
            q16 = gp.tile([P, 6, TR], f16)
            h = gp.tile([P, 3, TR], f32)
            w2 = gp.tile([P, 2, TR], f32)
            nc.vector.tensor_add(h16[:, 0:13], sel[:, 0:13], sel[:, 13:26])
            nc.vector.tensor_add(q16[:, 0:6], h16[:, 0:6], h16[:, 6:12])
            nc.vector.tensor_add(h[:, 0:3], q16[:, 0:3], q16[:, 3:6])
            nc.vector.tensor_add(w2[:, 0:1], h[:, 0:1], h[:, 1:2])
            nc.vector.tensor_add(w2[:, 1:2], w2[:, 0:1], h[:, 2:3])
            nc.vector.tensor_add(h[:, 0:1], w2[:, 1:2], h16[:, 12:13])
            # epilogue, fused across tiles: tot = tree + psum (one op),
            # then per-tile ACT square-accumulate, one final add, 4 DMAs.
            tot_all = sp.tile([P, NT, ROW], f32, tag="tot")
            nc.vector.tensor_add(
                tot_all[:, :, :], h[:, 0, :], psum_all[:, :, :]
            )
            acc_all = sp.tile([P, NT], f32, tag="acc")
            scr0 = sp.tile([P, K], f32, tag="scr0")
            scr1 = sp.tile([P, K], f32, tag="scr1")
            scr2 = sp.tile([P, K], f32, tag="scr2")
            scr3 = sp.tile([P, K], f32, tag="scr3")
            for t, scr in enumerate((scr0, scr1, scr2, scr3)):
                nc.scalar.activation(
                    out=scr[:],
                    in_=tot_all[:, t, 0:K],
                    func=mybir.ActivationFunctionType.Square,
                    scale=0.7071067811865476,
                    accum_out=acc_all[:, t : t + 1],
                )
            o_all = sp.tile([P, NT], f32, tag="o")
            nc.vector.tensor_add(o_all[:], acc_all[:], tot_all[:, :, K])
            try:
                ytgt = y_d[:, 0].rearrange("(t p) -> p t", p=P)
                nc.sync.dma_start(ytgt, o_all[:])
            except Exception:
                for t in range(NT):
                    nc.sync.dma_start(
                        y_d[t * P : (t + 1) * P, :], o_all[:, t : t + 1]
                    )
    nc.compile()
    return nc


def _prepare(dense, sparse, w0, w, V):
    sp = sparse.astype(np.int64)  # [B, 26] category per field
    pair = (sp >> 1).astype(np.int16)  # [B, 26] pair index < 20000
    parity = (sp & 1).astype(np.float16)

    Vt = np.ascontiguousarray(V.T)  # [F, K] f32
    Vs = Vt[NUM_DENSE:]  # sparse-field rows [26*40000, K]
    ws = w[NUM_DENSE:, 0]
    rows = np.zeros((NUM_SPARSE * FEAT_NUM, ROWE), dtype=np.float16)
    rows[:, :K] = Vs.astype(np.float16)
    rows[:, K] = (ws - 0.5 * (Vs * Vs).sum(axis=1)).astype(np.float16)
    # pair layout [even | odd - even]: the on-chip parity select becomes
    # sel = even + m*delta (2 ops instead of 3)
    pairs = rows.reshape(NUM_SPARSE * PAIRS, 2, ROWE)
    tablep = np.empty((NUM_SPARSE * PAIRS, PAIR_E), dtype=np.float16)
    tablep[:, :ROWE] = pairs[:, 0]
    tablep[:, ROWE:] = pairs[:, 1] - pairs[:, 0]

    lhs = np.concatenate(
        [dense.T, dense.T**2, np.ones((1, BATCH), np.float32)], axis=0
    ).astype(np.float32)  # [27, BATCH]
    rhs = np.zeros((CDIM, ROW), dtype=np.float32)
    rhs[0:NUM_DENSE, 0:K] = Vt[:NUM_DENSE]
    rhs[0:NUM_DENSE, K] = w[:NUM_DENSE, 0]
    rhs[NUM_DENSE : 2 * NUM_DENSE, K] = -0.5 * (Vt[:NUM_DENSE] ** 2).sum(axis=1)
    rhs[2 * NUM_DENSE, K] = np.asarray(w0).reshape(-1)[0]
    return pair, parity, tablep, lhs, rhs


def kernel(dense_inputs, sparse_inputs, w0, w, V):
    dense = np.asarray(dense_inputs, dtype=np.float32)
    sparse = np.asarray(sparse_inputs)
    w0 = np.asarray(w0, dtype=np.float32)
    w = np.asarray(w, dtype=np.float32)
    V = np.asarray(V, dtype=np.float32)

    if not _nc_cache:
        _nc_cache.append(_build())
    nc = _nc_cache[0]

    pair, parity, tablep, lhs, rhs = _prepare(dense, sparse, w0, w, V)

    in_maps = []
    for c in range(N_CORES):
        rows = slice(c * BPC, (c + 1) * BPC)
        pair_c = pair[rows]  # [512, 26]
        # idx16[p, (s*NT+t)*CWT + cw] = pair_c[t*128 + cw*16 + p%16, s],
        # 16-row groups replicated across the 8 Q7 cores.
        wrapped = pair_c.reshape(NT, CWT, 16, NUM_SPARSE).transpose(
            2, 3, 0, 1
        )  # [16, 26, NT, CWT]
        idx16 = np.tile(wrapped.reshape(16, NUM_SPARSE * NT * CWT), (8, 1))
        # par[p, ((t*26)+s)*ROW + e] = parity of batch row t*128+p, field s
        # (mask pre-expanded along the ROW dim for contiguous DVE reads)
        par_c = parity[rows].reshape(NT, P, NUM_SPARSE).transpose(1, 0, 2)
        par_exp = np.repeat(
            par_c.reshape(P, NT, NUM_SPARSE, 1), ROW, axis=3
        )  # [128, NT, 26, ROW]
        in_maps.append(
            {
                "tablep": tablep,
                "idx": np.ascontiguousarray(idx16),
                "par": np.ascontiguousarray(
                    par_exp.reshape(P, NT * NUM_SPARSE * ROW)
                ),
                "lhs": np.ascontiguousarray(lhs[:, rows]),
                "rhs": rhs,
            }
        )
    res = bass_utils.run_bass_kernel_spmd(
        nc, in_maps, core_ids=list(range(N_CORES)), trace=TRACE
    )
    LAST["res"] = res
    out = np.concatenate([res.results[c]["y"] for c in range(N_CORES)], axis=0)
    return out.astype(np.float32)
